# revision 5
# baseline (speedup 1.0000x reference)
"""Multi-task MoE routing (nn_CGC_69836168233304) on 8 TRN2 NeuronCores.

Reference math:
  h[g,e] = relu(x @ W[g,e] + b[g,e])                   12 experts (3 groups x 4)
  sel_t  = softmax(x @ Wg[t] + bg[t])   over 8 cols    t in {0,1}
  sel_s  = softmax(x @ Wgs + bgs)       over 12 cols
  out_t  = sum_m sel_t[:,m] * concat(h[t], h[2])[m]    t in {0,1}
  out_s  = sum_m sel_s[:,m] * concat(h[0],h[1],h[2])[m]

Sharding: data-parallel over batch B=16384 -> 2048 rows/core; every core holds
all 12 expert weights (streamed from HBM per O-slice) and produces its batch
shard of all three outputs; host concatenates shards (no collectives needed).

Per-core kernel mapping (Bass/Tile):
  - x is staged TRANSPOSED (xT [D, BC]) and cast to fp16 on host so the
    contraction dim D lands on SBUF partitions for the PE.
  - matmul tiles: out rows (batch) on PSUM partitions, O sliced 512/bank;
    K=1024 contracted as 8 chained K=128 matmuls accumulating in PSUM.
  - expert bias enters PSUM via one K=1 ones-vector matmul (start=False).
  - relu on the scalar engine (ACT), PSUM -> SBUF.
  - gating: one fused [D,28] gate matmul + segment softmax (exp on ACT,
    reduce/reciprocal/scale on DVE); combine = per-partition-scalar fused
    multiply-add (scalar_tensor_tensor) into three SBUF accumulators.
  - fp16 matmul inputs (full PE rate, exact products into fp32 PSUM), fp32
    everywhere after PSUM: measured |err|_max/scale ~ 3e-4 vs fp32 reference.
"""

import numpy as np

import concourse.bacc as bacc
import concourse.mybir as mybir
import concourse.tile as tile
from concourse.bass_utils import run_bass_kernel_spmd

F32 = mybir.dt.float32
F16 = mybir.dt.float16

N_CORES = 8
B, D, O = 16384, 1024, 1024
BC = B // N_CORES
NE = 12  # experts
NG = 28  # gate columns: 8 (task0) + 8 (task1) + 12 (shared)
SEGS = ((0, 8), (8, 16), (16, 28))
G = 8  # batch tiles per accumulator group
OSL = 512  # output-column slice per PSUM bank


def _contribs(e):
    """(out_k, sel_col) pairs for expert e. Gate col order: t0=[g0e0..3,g2e0..3],
    t1=[g1e0..3,g2e0..3], shared=[g0,g1,g2]."""
    if e < 4:
        return [(0, e), (2, 16 + e)]
    if e < 8:
        return [(1, 8 + (e - 4)), (2, 20 + (e - 4))]
    return [(0, 4 + (e - 8)), (1, 12 + (e - 8)), (2, 24 + (e - 8))]


def _build(reps=1):
    KB = D // 128
    N_BT = BC // 128
    N_GRP = N_BT // G
    N_OSL = O // OSL
    dt = F16

    nc = bacc.Bacc("TRN2", target_bir_lowering=False, debug=False)

    xT_d = nc.dram_tensor("xT", [D, BC], dt, kind="ExternalInput")
    w_d = nc.dram_tensor("We", [NE, D, O], dt, kind="ExternalInput")
    b_d = nc.dram_tensor("Be", [1, NE, O], dt, kind="ExternalInput")
    wg_d = nc.dram_tensor("Wgc", [D, NG], dt, kind="ExternalInput")
    bg_d = nc.dram_tensor("bgc", [1, NG], dt, kind="ExternalInput")
    out_d = [
        nc.dram_tensor(f"out{k}", [BC, O], F32, kind="ExternalOutput")
        for k in range(3)
    ]

    with tile.TileContext(nc) as tc:
        with (
            tc.tile_pool(name="big", bufs=1) as big,
            tc.tile_pool(name="wpool", bufs=2) as wpool,
            tc.tile_pool(name="accp", bufs=1) as accp,
            tc.tile_pool(name="tmpp", bufs=4) as tmpp,
            tc.tile_pool(name="gatep", bufs=2) as gatep,
            tc.tile_pool(name="psum", bufs=6, space="PSUM") as psum,
            tc.tile_pool(name="psumg", bufs=2, space="PSUM") as psumg,
        ):
            # --- resident staging ---
            x_sb = big.tile([128, KB, BC], dt)
            nc.sync.dma_start(
                x_sb[:], xT_d.ap().rearrange("(k p) b -> p k b", p=128)
            )
            wg_sb = big.tile([128, KB, NG], dt)
            nc.sync.dma_start(
                wg_sb[:], wg_d.ap().rearrange("(k p) g -> p k g", p=128)
            )
            bg_sb = big.tile([1, NG], dt)
            nc.sync.dma_start(bg_sb[:], bg_d.ap())
            be_sb = big.tile([1, NE, O], dt)
            nc.sync.dma_start(be_sb[:], b_d.ap())
            ones_sb = big.tile([1, 128], dt)
            nc.vector.memset(ones_sb[:], 1.0)

            sel_sb = big.tile([128, N_BT, NG], F32)

            for _rep in range(reps):
                _emit_body(
                    nc, tc, x_sb, wg_sb, bg_sb, be_sb, ones_sb, sel_sb,
                    wpool, accp, tmpp, gatep, psum, psumg, w_d, out_d,
                    KB, N_BT, N_GRP, N_OSL, dt,
                )

    nc.compile()
    return nc


def _emit_body(
    nc, tc, x_sb, wg_sb, bg_sb, be_sb, ones_sb, sel_sb,
    wpool, accp, tmpp, gatep, psum, psumg, w_d, out_d,
    KB, N_BT, N_GRP, N_OSL, dt,
):
    if True:
        if True:
            # --- gates: sel = segment softmax(x @ Wg_cat + bg_cat) ---
            for bt in range(N_BT):
                pg = psumg.tile([128, NG], F32)
                for k in range(KB):
                    nc.tensor.matmul(
                        pg[:],
                        x_sb[:, k, bt * 128 : (bt + 1) * 128],
                        wg_sb[:, k, :],
                        start=(k == 0),
                        stop=False,
                    )
                nc.tensor.matmul(
                    pg[:], ones_sb[:], bg_sb[:], start=False, stop=True
                )
                et = gatep.tile([128, NG], F32)
                nc.scalar.activation(
                    et[:], pg[:], mybir.ActivationFunctionType.Exp
                )
                for s0, s1 in SEGS:
                    den = gatep.tile([128, 1], F32, tag="den")
                    nc.vector.tensor_reduce(
                        den[:], et[:, s0:s1], mybir.AxisListType.X,
                        mybir.AluOpType.add,
                    )
                    rden = gatep.tile([128, 1], F32, tag="rden")
                    nc.vector.reciprocal(rden[:], den[:])
                    nc.vector.tensor_scalar(
                        sel_sb[:, bt, s0:s1], et[:, s0:s1], rden[:], None,
                        mybir.AluOpType.mult,
                    )

            # --- experts + gated accumulation ---
            for osl in range(N_OSL):
                o0 = osl * OSL
                for grp in range(N_GRP):
                    bt0 = grp * G
                    accs = {}
                    for e in range(NE):
                        w_sb = wpool.tile([128, KB, OSL], dt, tag="w")
                        nc.sync.dma_start(
                            w_sb[:],
                            w_d.ap()[e, :, o0 : o0 + OSL].rearrange(
                                "(k p) o -> p k o", p=128
                            ),
                        )
                        for bt in range(bt0, bt0 + G):
                            ps = psum.tile([128, OSL], F32)
                            for k in range(KB):
                                nc.tensor.matmul(
                                    ps[:],
                                    x_sb[:, k, bt * 128 : (bt + 1) * 128],
                                    w_sb[:, k, :],
                                    start=(k == 0),
                                    stop=False,
                                )
                            nc.tensor.matmul(
                                ps[:], ones_sb[:], be_sb[:, e, o0 : o0 + OSL],
                                start=False, stop=True,
                            )
                            tmp = tmpp.tile([128, OSL], F32)
                            nc.scalar.activation(
                                tmp[:], ps[:], mybir.ActivationFunctionType.Relu
                            )
                            for k, col in _contribs(e):
                                sc = sel_sb[:, bt, col : col + 1]
                                key = (k, bt)
                                if key not in accs:
                                    a = accp.tile(
                                        [128, OSL], F32, tag=f"acc{k}_{bt - bt0}"
                                    )
                                    accs[key] = a
                                    nc.vector.tensor_scalar(
                                        a[:], tmp[:], sc, None,
                                        mybir.AluOpType.mult,
                                    )
                                else:
                                    a = accs[key]
                                    nc.vector.scalar_tensor_tensor(
                                        a[:], tmp[:], sc, a[:],
                                        mybir.AluOpType.mult,
                                        mybir.AluOpType.add,
                                    )
                    for (k, bt), a in accs.items():
                        nc.sync.dma_start(
                            out_d[k].ap()[bt * 128 : (bt + 1) * 128, o0 : o0 + OSL],
                            a[:],
                        )


_NC_CACHE = None


def make_in_maps(x, W, b, Wg, bg, Wgs, bgs):
    x = np.asarray(x, dtype=np.float32)
    np_dt = np.float16
    shared = {
        "We": np.ascontiguousarray(np.asarray(W).reshape(NE, D, O)).astype(np_dt),
        "Be": np.asarray(b).reshape(1, NE, O).astype(np_dt),
        "Wgc": np.concatenate(
            [np.asarray(Wg)[0], np.asarray(Wg)[1], np.asarray(Wgs)], axis=1
        ).astype(np_dt),
        "bgc": np.concatenate(
            [np.asarray(bg)[0], np.asarray(bg)[1], np.asarray(bgs)]
        )[None, :].astype(np_dt),
    }
    in_maps = []
    for c in range(N_CORES):
        m = dict(shared)
        m["xT"] = np.ascontiguousarray(
            x[c * BC : (c + 1) * BC].T
        ).astype(np_dt)
        in_maps.append(m)
    return in_maps


def _gather(res):
    return tuple(
        np.concatenate(
            [res.results[c][f"out{k}"] for c in range(N_CORES)], axis=0
        ).astype(np.float32)
        for k in range(3)
    )


def kernel(x, W, b, Wg, bg, Wgs, bgs):
    global _NC_CACHE
    if _NC_CACHE is None:
        _NC_CACHE = _build()
    nc = _NC_CACHE

    in_maps = make_in_maps(x, W, b, Wg, bg, Wgs, bgs)
    res = run_bass_kernel_spmd(nc, in_maps, list(range(N_CORES)))
    return _gather(res)

